# revision 30
# baseline (speedup 1.0000x reference)
"""Multi-head self-attention (B=8, S=1024, E=1024, H=16) on 8 TRN2 cores.

Sharding: data-parallel on batch — core i computes batch i, all 16 heads.
Device computes pure causal attention (bias folded into q/k/v); rows q >= l[b]
are zeroed on the host (causal & q<l implies k<l, so the padding mask is
redundant for valid rows).
"""

import sys

sys.path.insert(0, "/opt/trn_rl_repo")

import numpy as np
import ml_dtypes

import concourse.bass as bass
import concourse.bacc as bacc
import concourse.mybir as mybir
import concourse.tile as tile
from concourse.bass import ds, ts
from concourse.bass_utils import run_bass_kernel_spmd

P = 128
B, S, E, H = 8, 1024, 1024, 16
DH = E // H  # 64
NT = S // P  # 8
F32 = mybir.dt.float32
BF16 = mybir.dt.bfloat16
F32R = mybir.dt.float32r

_cached = None


def _build_program():
    nc = bacc.Bacc(None, target_bir_lowering=False)

    xT = nc.dram_tensor("xT", [E, S], BF16, kind="ExternalInput")[:]
    wT = nc.dram_tensor("wT", [E, 3 * E], BF16, kind="ExternalInput")[:]
    bqk = nc.dram_tensor("bqk", [P, 16], F32, kind="ExternalInput")[:]
    bv = nc.dram_tensor("bv", [1, E], BF16, kind="ExternalInput")[:]
    cm = nc.dram_tensor("cm", [P, P], BF16, kind="ExternalInput")[:]
    ones = nc.dram_tensor("ones", [1, P], BF16, kind="ExternalInput")[:]
    o = nc.dram_tensor("o", [S, E], F32, kind="ExternalOutput")[:]

    with tile.TileContext(nc) as tc:
        from contextlib import ExitStack

        with ExitStack() as ctx:
            sb = ctx.enter_context(tc.tile_pool(name="sb", bufs=1))
            xT_sb = sb.tile([P, NT, S], BF16)       # [e_p, e_t, s]
            qkT_sb = sb.tile([P, 16, S], BF16)      # [j_p, j_t, s] (8 Q tiles, 8 K tiles)
            vp_sb = sb.tile([P, NT, H, DH + 1], BF16)  # [s_p, s_t, h, d] + ones col
            out_sb = sb.tile([P, NT, E], F32)       # [q_p, t_q, j]
            bqk_sb = sb.tile([P, 16], F32)
            bv_sb = sb.tile([1, E], BF16)
            cm_sb = sb.tile([P, P], BF16)
            ones_sb = sb.tile([1, P], BF16)

            for e_t in range(NT):
                nc.sync.dma_start(
                    out=xT_sb[:, e_t, :], in_=xT[ds(e_t * P, P), :])
            nc.sync.dma_start(out=bqk_sb, in_=bqk)
            nc.sync.dma_start(out=bv_sb, in_=bv)
            nc.sync.dma_start(out=cm_sb, in_=cm)
            nc.sync.dma_start(out=ones_sb, in_=ones)
            nc.vector.memset(vp_sb[:, :, :, DH : DH + 1], 1.0)

            wblk_pool = ctx.enter_context(tc.tile_pool(name="wblk", bufs=3))
            qk_psum = ctx.enter_context(
                tc.tile_pool(name="qk_psum", bufs=2, space="PSUM"))

            def emit_qk(j_t):
                # qkT_sb[:, j_t, :] = (W_row_block @ x^T + bias), cast bf16
                wblk = wblk_pool.tile([P, NT, P], BF16)
                nc.sync.dma_start(
                    out=wblk,
                    in_=wT[:, ds(j_t * P, P)].rearrange("(t p) j -> p t j", p=P))
                for s_half in range(2):
                    ps = qk_psum.tile([P, 512], F32)
                    for e_t in range(NT):
                        nc.tensor.matmul(
                            ps,
                            lhsT=wblk[:, e_t, :],
                            rhs=xT_sb[:, e_t, ds(s_half * 512, 512)],
                            start=(e_t == 0),
                            stop=(e_t == NT - 1))
                    nc.scalar.activation(
                        out=qkT_sb[:, j_t, ds(s_half * 512, 512)],
                        in_=ps,
                        func=mybir.ActivationFunctionType.Identity,
                        bias=bqk_sb[:, ds(j_t, 1)],
                        scale=1.0)

            def emit_v(jv_half, wv_pool):
                # vp_sb[:, s_t, 8*jv_half:+8, 0:64] = x @ W_v_cols + bias
                wv = wv_pool.tile([P, NT, 512], BF16, name="wv")
                nc.sync.dma_start(
                    out=wv,
                    in_=wT[:, ds(2 * E + jv_half * 512, 512)].rearrange(
                        "(t p) j -> p t j", p=P))
                for s_t in range(NT):
                    ps = qk_psum.tile([P, 512], F32)
                    for e_t in range(NT):
                        nc.tensor.matmul(
                            ps,
                            lhsT=xT_sb[:, e_t, ts(s_t, P)],
                            rhs=wv[:, e_t, :],
                            start=(e_t == 0),
                            stop=False)
                    nc.tensor.matmul(
                        ps,
                        lhsT=ones_sb,
                        rhs=bv_sb[:, ds(jv_half * 512, 512)],
                        start=False,
                        stop=True)
                    nc.vector.tensor_copy(
                        out=vp_sb[:, s_t, ds(jv_half * 8, 8), 0:DH],
                        in_=ps.rearrange("p (h d) -> p h d", h=8))

            def emit_attn(hp):
                h0, h1 = 2 * hp, 2 * hp + 1
                eT = {h: eT_pool.tile([P, NT, S], BF16, name="eT")
                      for h in (h0, h1)}
                for t_k in range(NT):
                    q0 = t_k * P
                    for h, base in ((h0, 0), (h1, 64)):
                        ps = s_psum.tile([P, 1024], F32)
                        if t_k < 4:
                            chunks = [(q0, 512 - q0), (512, 512)]
                        else:
                            chunks = [(q0, S - q0)]
                        for (c0, cn) in chunks:
                            nc.tensor.matmul(
                                ps[:, ds(c0, cn)],
                                lhsT=qkT_sb[base:base + 64, 8 + hp, ts(t_k, P)],
                                rhs=qkT_sb[base:base + 64, hp, ds(c0, cn)],
                                start=True,
                                stop=True)
                        nc.scalar.activation(
                            out=eT[h][:, t_k, ds(q0, S - q0)],
                            in_=ps[:, ds(q0, S - q0)],
                            func=mybir.ActivationFunctionType.Exp,
                            scale=1.0 / 32.0)
                    for h in (h0, h1):
                        nc.vector.tensor_mul(
                            eT[h][:, t_k, ds(q0, P)],
                            eT[h][:, t_k, ds(q0, P)],
                            cm_sb)
                for h in (h0, h1):
                    for t_q in range(NT):
                        po = o_psum.tile([P, 512], F32)
                        for t_k in range(t_q + 1):
                            nc.tensor.matmul(
                                po[:, 0:DH + 1],
                                lhsT=eT[h][:, t_k, ts(t_q, P)],
                                rhs=vp_sb[:, t_k, h, :],
                                start=(t_k == 0),
                                stop=(t_k == t_q))
                        rec = rec_pool.tile([P, 1], F32)
                        nc.vector.reciprocal(rec, po[:, DH:DH + 1])
                        nc.vector.tensor_scalar_mul(
                            out_sb[:, t_q, ds(h * DH, DH)],
                            po[:, 0:DH],
                            rec)

            # Emission schedule: keep PE fed, overlap phase1 with phase2.
            emit_qk(0)       # Q pair 0
            emit_qk(8)       # K pair 0
            with tc.tile_pool(name="wv", bufs=2) as wv_pool:
                emit_v(0, wv_pool)   # heads 0-7
                emit_v(1, wv_pool)   # heads 8-15
            eT_pool = ctx.enter_context(tc.tile_pool(name="eT", bufs=3))
            rec_pool = ctx.enter_context(tc.tile_pool(name="rec", bufs=4))
            s_psum = ctx.enter_context(
                tc.tile_pool(name="s_psum", bufs=2, space="PSUM"))
            o_psum = ctx.enter_context(
                tc.tile_pool(name="o_psum", bufs=2, space="PSUM"))
            emit_qk(1)
            emit_qk(9)
            for hp in range(8):
                emit_attn(hp)
                if hp + 2 < 8:
                    emit_qk(hp + 2)
                    emit_qk(8 + hp + 2)
                for t_q in range(NT):
                    nc.sync.dma_start(
                        out=o[ts(t_q, P), ds(hp * P, P)],
                        in_=out_sb[:, t_q, ds(hp * P, P)])

    nc.compile()
    return nc


def _prepare_in_maps(x, l, W, b):
    wTc = np.ascontiguousarray(W.T.astype(ml_dtypes.bfloat16))
    bqk = np.ascontiguousarray(
        b[: 2 * E].astype(np.float32).reshape(16, P).T)
    bv = np.ascontiguousarray(
        b[2 * E :].astype(ml_dtypes.bfloat16).reshape(1, E))
    k_idx = np.arange(P)[:, None]
    q_idx = np.arange(P)[None, :]
    cm = (k_idx <= q_idx).astype(ml_dtypes.bfloat16)
    ones = np.ones((1, P), ml_dtypes.bfloat16)
    in_maps = []
    for bi in range(B):
        xTb = np.ascontiguousarray(x[bi].T.astype(ml_dtypes.bfloat16))
        in_maps.append(
            {"xT": xTb, "wT": wTc, "bqk": bqk, "bv": bv, "cm": cm,
             "ones": ones})
    return in_maps


def _run(x, l, W, b, trace=False):
    global _cached
    if _cached is None:
        _cached = _build_program()
    nc = _cached
    in_maps = _prepare_in_maps(x, l, W, b)
    res = run_bass_kernel_spmd(nc, in_maps, list(range(B)), trace=trace)
    out = np.stack([res.results[i]["o"] for i in range(B)]).astype(np.float32)
    lv = np.asarray(l).astype(np.int64)
    for bi in range(B):
        out[bi, int(lv[bi]) :, :] = 0.0
    return out, res.exec_time_ns


def kernel(x, l, W, b):
    out, _ = _run(x, l, W, b, trace=False)
    return out


# revision 32
# speedup vs baseline: 1.1968x; 1.1968x over previous
"""Multi-head self-attention (B=8, S=1024, E=1024, H=16) on 8 TRN2 cores.

Sharding: data-parallel on batch — core i computes batch i, all 16 heads.
Device computes pure causal attention (bias folded into q/k/v); rows q >= l[b]
are zeroed on the host (causal & q<l implies k<l, so the padding mask is
redundant for valid rows).
"""

import sys

sys.path.insert(0, "/opt/trn_rl_repo")

import numpy as np
import ml_dtypes

import concourse.bass as bass
import concourse.bacc as bacc
import concourse.mybir as mybir
import concourse.tile as tile
from concourse.bass import ds, ts
from concourse.bass_utils import run_bass_kernel_spmd

P = 128
B, S, E, H = 8, 1024, 1024, 16
DH = E // H  # 64
NT = S // P  # 8
F32 = mybir.dt.float32
BF16 = mybir.dt.bfloat16
F32R = mybir.dt.float32r

_cached = None


def _build_program():
    nc = bacc.Bacc(None, target_bir_lowering=False)

    xT = nc.dram_tensor("xT", [E, S], BF16, kind="ExternalInput")[:]
    wT = nc.dram_tensor("wT", [E, 3 * E], BF16, kind="ExternalInput")[:]
    bqk = nc.dram_tensor("bqk", [P, 16], F32, kind="ExternalInput")[:]
    bv = nc.dram_tensor("bv", [1, E], BF16, kind="ExternalInput")[:]
    cm = nc.dram_tensor("cm", [P, P], BF16, kind="ExternalInput")[:]
    ones = nc.dram_tensor("ones", [1, P], BF16, kind="ExternalInput")[:]
    o = nc.dram_tensor("o", [S, E], F32, kind="ExternalOutput")[:]

    with tile.TileContext(nc) as tc:
        from contextlib import ExitStack

        with ExitStack() as ctx:
            sb = ctx.enter_context(tc.tile_pool(name="sb", bufs=1))
            xT_sb = sb.tile([P, NT, S], BF16)       # [e_p, e_t, s]
            qkT_sb = sb.tile([P, 16, S], BF16)      # [j_p, j_t, s] (8 Q tiles, 8 K tiles)
            vp_sb = sb.tile([P, NT, H, DH + 1], BF16)  # [s_p, s_t, h, d] + ones col
            out_sb = sb.tile([P, NT, E], F32)       # [q_p, t_q, j]
            bqk_sb = sb.tile([P, 16], F32)
            bv_sb = sb.tile([1, E], BF16)
            cm_sb = sb.tile([P, P], BF16)
            ones_sb = sb.tile([1, P], BF16)

            for e_t in range(NT):
                nc.sync.dma_start(
                    out=xT_sb[:, e_t, :], in_=xT[ds(e_t * P, P), :])
            nc.sync.dma_start(out=bqk_sb, in_=bqk)
            nc.sync.dma_start(out=bv_sb, in_=bv)
            nc.sync.dma_start(out=cm_sb, in_=cm)
            nc.sync.dma_start(out=ones_sb, in_=ones)
            nc.vector.memset(vp_sb[:, :, :, DH : DH + 1], 1.0)

            wblk_pool = ctx.enter_context(tc.tile_pool(name="wblk", bufs=3))
            qk_psum = ctx.enter_context(
                tc.tile_pool(name="qk_psum", bufs=2, space="PSUM"))

            def emit_qk(j_t):
                # qkT_sb[:, j_t, :] = (W_row_block @ x^T + bias), cast bf16
                wblk = wblk_pool.tile([P, NT, P], BF16)
                nc.sync.dma_start(
                    out=wblk,
                    in_=wT[:, ds(j_t * P, P)].rearrange("(t p) j -> p t j", p=P))
                for s_half in range(2):
                    ps = qk_psum.tile([P, 512], F32)
                    for e_t in range(NT):
                        nc.tensor.matmul(
                            ps,
                            lhsT=wblk[:, e_t, :],
                            rhs=xT_sb[:, e_t, ds(s_half * 512, 512)],
                            start=(e_t == 0),
                            stop=(e_t == NT - 1))
                    nc.scalar.activation(
                        out=qkT_sb[:, j_t, ds(s_half * 512, 512)],
                        in_=ps,
                        func=mybir.ActivationFunctionType.Identity,
                        bias=bqk_sb[:, ds(j_t, 1)],
                        scale=1.0)

            def emit_v(jv_half, wv_pool):
                # vp_sb[:, s_t, 8*jv_half:+8, 0:64] = x @ W_v_cols + bias
                wv = wv_pool.tile([P, NT, 512], BF16, name="wv")
                nc.sync.dma_start(
                    out=wv,
                    in_=wT[:, ds(2 * E + jv_half * 512, 512)].rearrange(
                        "(t p) j -> p t j", p=P))
                for s_t in range(NT):
                    ps = qk_psum.tile([P, 512], F32)
                    for e_t in range(NT):
                        nc.tensor.matmul(
                            ps,
                            lhsT=xT_sb[:, e_t, ts(s_t, P)],
                            rhs=wv[:, e_t, :],
                            start=(e_t == 0),
                            stop=False)
                    nc.tensor.matmul(
                        ps,
                        lhsT=ones_sb,
                        rhs=bv_sb[:, ds(jv_half * 512, 512)],
                        start=False,
                        stop=True)
                    nc.vector.tensor_copy(
                        out=vp_sb[:, s_t, ds(jv_half * 8, 8), 0:DH],
                        in_=ps.rearrange("p (h d) -> p h d", h=8))

            def emit_attn(hp):
                h0, h1 = 2 * hp, 2 * hp + 1
                eT = {h: eT_pool.tile([P, NT, S], BF16, name="eT")
                      for h in (h0, h1)}
                for t_k in range(NT):
                    q0 = t_k * P
                    if t_k < 4:
                        chunks = [(q0, 512 - q0), (512, 512)]
                    else:
                        chunks = [(q0, S - q0)]
                    for (c0, cn) in chunks:
                        for h, base in ((h0, 0), (h1, 64)):
                            ps = s_psum.tile([P, 512], F32)
                            nc.tensor.matmul(
                                ps[:, 0:cn],
                                lhsT=qkT_sb[base:base + 64, 8 + hp, ts(t_k, P)],
                                rhs=qkT_sb[base:base + 64, hp, ds(c0, cn)],
                                start=True,
                                stop=True)
                            nc.scalar.activation(
                                out=eT[h][:, t_k, ds(c0, cn)],
                                in_=ps[:, 0:cn],
                                func=mybir.ActivationFunctionType.Exp,
                                scale=1.0 / 32.0)
                    for h in (h0, h1):
                        nc.vector.tensor_mul(
                            eT[h][:, t_k, ds(q0, P)],
                            eT[h][:, t_k, ds(q0, P)],
                            cm_sb)
                for h in (h0, h1):
                    for t_q in range(NT):
                        po = o_psum.tile([P, 512], F32)
                        for t_k in range(t_q + 1):
                            nc.tensor.matmul(
                                po[:, 0:DH + 1],
                                lhsT=eT[h][:, t_k, ts(t_q, P)],
                                rhs=vp_sb[:, t_k, h, :],
                                start=(t_k == 0),
                                stop=(t_k == t_q))
                        rec = rec_pool.tile([P, 1], F32)
                        nc.vector.reciprocal(rec, po[:, DH:DH + 1])
                        nc.vector.tensor_scalar_mul(
                            out_sb[:, t_q, ds(h * DH, DH)],
                            po[:, 0:DH],
                            rec)

            # Emission schedule: keep PE fed, overlap phase1 with phase2.
            emit_qk(0)       # Q pair 0
            emit_qk(8)       # K pair 0
            with tc.tile_pool(name="wv", bufs=2) as wv_pool:
                emit_v(0, wv_pool)   # heads 0-7
                emit_v(1, wv_pool)   # heads 8-15
            eT_pool = ctx.enter_context(tc.tile_pool(name="eT", bufs=3))
            rec_pool = ctx.enter_context(tc.tile_pool(name="rec", bufs=4))
            s_psum = ctx.enter_context(
                tc.tile_pool(name="s_psum", bufs=4, space="PSUM"))
            o_psum = ctx.enter_context(
                tc.tile_pool(name="o_psum", bufs=2, space="PSUM"))
            emit_qk(1)
            emit_qk(9)
            for hp in range(8):
                emit_attn(hp)
                if hp + 2 < 8:
                    emit_qk(hp + 2)
                    emit_qk(8 + hp + 2)
                for t_q in range(NT):
                    nc.sync.dma_start(
                        out=o[ts(t_q, P), ds(hp * P, P)],
                        in_=out_sb[:, t_q, ds(hp * P, P)])

    nc.compile()
    return nc


def _prepare_in_maps(x, l, W, b):
    wTc = np.ascontiguousarray(W.T.astype(ml_dtypes.bfloat16))
    bqk = np.ascontiguousarray(
        b[: 2 * E].astype(np.float32).reshape(16, P).T)
    bv = np.ascontiguousarray(
        b[2 * E :].astype(ml_dtypes.bfloat16).reshape(1, E))
    k_idx = np.arange(P)[:, None]
    q_idx = np.arange(P)[None, :]
    cm = (k_idx <= q_idx).astype(ml_dtypes.bfloat16)
    ones = np.ones((1, P), ml_dtypes.bfloat16)
    in_maps = []
    for bi in range(B):
        xTb = np.ascontiguousarray(x[bi].T.astype(ml_dtypes.bfloat16))
        in_maps.append(
            {"xT": xTb, "wT": wTc, "bqk": bqk, "bv": bv, "cm": cm,
             "ones": ones})
    return in_maps


def _run(x, l, W, b, trace=False):
    global _cached
    if _cached is None:
        _cached = _build_program()
    nc = _cached
    in_maps = _prepare_in_maps(x, l, W, b)
    res = run_bass_kernel_spmd(nc, in_maps, list(range(B)), trace=trace)
    out = np.stack([res.results[i]["o"] for i in range(B)]).astype(np.float32)
    lv = np.asarray(l).astype(np.int64)
    for bi in range(B):
        out[bi, int(lv[bi]) :, :] = 0.0
    return out, res.exec_time_ns


def kernel(x, l, W, b):
    out, _ = _run(x, l, W, b, trace=False)
    return out
